# revision 1
# baseline (speedup 1.0000x reference)
"""Trainium2 Bass kernel for nn_Normalizer (annealed top-k masking normalizer).

Math notes (derived from the reference):
  - The reference loop maintains b = -relu(score+a), so score+b = min(score,-a)
    and each iteration is s_t = sum(exp(min(score,-a)/theta_t)).
  - In exp-space with F_t = exp(sm/theta_t) (sm = masked score, unclipped):
        s_t = sum(min(F_t, cv_t)),   cv_t = exp(-a_{t-1}/theta_t)
    and since a_t = theta_t*log(k/s_t'), the clip level updates with plain
    arithmetic:  cv_t = (s_{t-1}'/k)^(theta_{t-1}/theta_t)  -- no log/exp.
  - For t>=8 theta_t == 0.3 is constant, so E = exp(sm/0.3) is computed once
    and each iteration is one fused DVE min+row-sum; the exponent ratio is 1
    so cv_t = s'/k directly.
  - gamma = exp(min(sm + a, 0)/0.3) = min(exp(sm/0.3) * k/s_19', 1).
  - Errors injected at iteration t decay by ~0.55 per subsequent iteration, so
    the t=0..7 varying-theta phase runs on a 1/8 column subsample (chunks of 16
    columns every 128, DMA-friendly) with a subsample-consistent k; the 12
    constant-theta iterations run full width.  Validated vs. the f32 reference
    at <2e-3 max elementwise relative error.

The only ACT function used is Exp (the tiny per-row clip updates use DVE pow),
so there is exactly one activation-table load in the whole kernel.

Sharding: pure row-parallel, 4096 rows -> 8 cores x 512 rows.
Each core processes 4 tiles of [128 rows, 8192 cols].
"""

import os
import sys

import numpy as np

try:
    import concourse.bass as bass
except ImportError:
    sys.path.insert(0, "/opt/trn_rl_repo")
    import concourse.bass as bass  # noqa: F401

import ml_dtypes

import concourse.bacc as bacc
import concourse.tile as tile
from concourse import mybir
from concourse.bass_utils import run_bass_kernel_spmd

F32 = mybir.dt.float32
BF16 = mybir.dt.bfloat16
A = mybir.AluOpType
AF = mybir.ActivationFunctionType

# Problem constants
THETA, THETA0, T_ITERS, BETA, P_FRAC = 0.3, 4.0, 20, 0.7, 0.1
BSZ, SEQ = 4096, 8192
N_CORES = 8
ROWS_PER_CORE = BSZ // N_CORES          # 512
P = 128                                  # partitions
N_TILES = ROWS_PER_CORE // P             # 4
CHUNK = 16                               # subsample: 16 cols every 128
CHUNK_EVERY = 128
N_CHUNKS = SEQ // CHUNK_EVERY            # 64
SUB = N_CHUNKS * CHUNK                   # 1024
BIG = 1.0e30

THETAS = [max(BETA**t * THETA0, THETA) for t in range(T_ITERS)]
N_SUB_ITERS = int(os.environ.get("NORM_SUB_ITERS", "12"))
N_FULL_ITERS = int(os.environ.get("NORM_FULL_ITERS", "12"))
SUB_CONTIG = os.environ.get("NORM_SUB_CONTIG", "0") == "1"
SKIP_STT = os.environ.get("NORM_SKIP_STT", "0") == "1"


def _chunk_view(ap):
    """[P, SEQ] access pattern -> [P, N_CHUNKS, CHUNK] subsample view."""
    return ap.rearrange("p (c l) -> p c l", l=CHUNK_EVERY)[:, :, 0:CHUNK]


def build_kernel(loop_n: int = 1):
    nc = bacc.Bacc("TRN2", target_bir_lowering=False, debug=False,
                   num_devices=N_CORES)
    score_d = nc.dram_tensor("score", [ROWS_PER_CORE, SEQ], F32,
                             kind="ExternalInput")
    maskbf_d = nc.dram_tensor("maskbf", [ROWS_PER_CORE, SEQ], BF16,
                              kind="ExternalInput")
    gamma_d = nc.dram_tensor("gamma", [ROWS_PER_CORE, SEQ], F32,
                             kind="ExternalOutput")

    with tile.TileContext(nc) as tc:
        import contextlib
        loop_cm = tc.For_i(0, loop_n, 1) if loop_n > 1 else \
            contextlib.nullcontext()
        with (
            loop_cm,
            tc.tile_pool(name="smp", bufs=2) as smp,
            tc.tile_pool(name="ep", bufs=2) as ep,
            tc.tile_pool(name="mpp", bufs=2) as mpp,
            tc.tile_pool(name="junkp", bufs=2) as junkp,
            tc.tile_pool(name="ssubp", bufs=2) as ssubp,
            tc.tile_pool(name="psubp", bufs=2) as psubp,
            tc.tile_pool(name="esubp", bufs=2) as esubp,
            tc.tile_pool(name="sjunkp", bufs=2) as sjunkp,
            tc.tile_pool(name="scalars", bufs=4 * N_TILES) as scalars,
        ):
            for j in range(N_TILES):
                r0 = j * P
                # ---- DMAs ------------------------------------------------
                st = ssubp.tile([P, SUB], F32, tag="ssub")
                pt = psubp.tile([P, SUB], BF16, tag="psub")
                if SUB_CONTIG:
                    nc.sync.dma_start(out=st[:],
                                      in_=score_d.ap()[r0:r0 + P, 0:SUB])
                    nc.sync.dma_start(out=pt[:],
                                      in_=maskbf_d.ap()[r0:r0 + P, 0:SUB])
                else:
                    nc.sync.dma_start(
                        out=st[:].rearrange("p (c l) -> p c l", l=CHUNK),
                        in_=_chunk_view(score_d.ap()[r0:r0 + P, :]))
                    nc.sync.dma_start(
                        out=pt[:].rearrange("p (c l) -> p c l", l=CHUNK),
                        in_=_chunk_view(maskbf_d.ap()[r0:r0 + P, :]))
                mp = mpp.tile([P, SEQ], BF16, tag="mp")
                nc.sync.dma_start(out=mp[:], in_=maskbf_d.ap()[r0:r0 + P, :])
                sm = smp.tile([P, SEQ], F32, tag="sm")
                nc.sync.dma_start(out=sm[:], in_=score_d.ap()[r0:r0 + P, :])

                # ---- subsample: E_sub and k_sub --------------------------
                nc.vector.scalar_tensor_tensor(
                    out=st[:], in0=pt[:], scalar=0.0, in1=st[:],
                    op0=A.add, op1=A.add)
                sj = sjunkp.tile([P, SUB], BF16, tag="sjunk")
                cnt_s = scalars.tile([P, 1], F32, tag="cnts")
                nc.vector.tensor_scalar(out=sj[:], in0=pt[:],
                                        scalar1=0.0, scalar2=None,
                                        op0=A.is_equal, op1=A.add,
                                        accum_out=cnt_s[:])
                ks_t = scalars.tile([P, 1], F32, tag="ks")
                nc.vector.tensor_scalar_mul(out=ks_t[:], in0=cnt_s[:],
                                            scalar1=P_FRAC)
                rks_t = scalars.tile([P, 1], F32, tag="rks")
                nc.vector.reciprocal(out=rks_t[:], in_=ks_t[:])
                es_t = esubp.tile([P, SUB], BF16, tag="esub")
                nc.scalar.activation(out=es_t[:], in_=st[:], func=AF.Exp,
                                     scale=1.0 / THETA)

                # ---- full-width setup ------------------------------------
                if not SKIP_STT:
                    nc.vector.scalar_tensor_tensor(
                        out=sm[:], in0=mp[:], scalar=0.0, in1=sm[:],
                        op0=A.add, op1=A.add)
                junk = junkp.tile([P, SEQ], BF16, tag="junk")
                cnt = scalars.tile([P, 1], F32, tag="cnt")
                nc.vector.tensor_scalar(out=junk[:], in0=mp[:], scalar1=0.0,
                                        scalar2=None, op0=A.is_equal,
                                        op1=A.add, accum_out=cnt[:])
                k_t = scalars.tile([P, 1], F32, tag="k")
                nc.vector.tensor_scalar_mul(out=k_t[:], in0=cnt[:],
                                            scalar1=P_FRAC)
                rk = scalars.tile([P, 1], F32, tag="rk")
                nc.vector.reciprocal(out=rk[:], in_=k_t[:])
                # E = exp(sm/0.3) bf16;  G = exp(sm/0.3) f32 in place over sm
                e_t = ep.tile([P, SEQ], BF16, tag="E")
                nc.scalar.activation(out=e_t[:], in_=sm[:], func=AF.Exp,
                                     scale=1.0 / THETA)
                nc.scalar.activation(out=sm[:], in_=sm[:], func=AF.Exp,
                                     scale=1.0 / THETA)

                # ---- converge c on the subsample (from far above) --------
                c_t = None
                for t in range(N_SUB_ITERS):
                    sj = sjunkp.tile([P, SUB], BF16, tag="sjunk")
                    s_t = scalars.tile([P, 1], F32, tag="s")
                    if c_t is None:
                        nc.vector.tensor_scalar(out=sj[:], in0=es_t[:],
                                                scalar1=BIG, scalar2=None,
                                                op0=A.min, op1=A.add,
                                                accum_out=s_t[:])
                    else:
                        nc.vector.tensor_scalar(out=sj[:], in0=es_t[:],
                                                scalar1=c_t[:], scalar2=None,
                                                op0=A.min, op1=A.add,
                                                accum_out=s_t[:])
                    c_t = scalars.tile([P, 1], F32, tag="c")
                    nc.vector.tensor_scalar(out=c_t[:], in0=s_t[:],
                                            scalar1=1e-20, scalar2=rks_t[:],
                                            op0=A.add, op1=A.mult)

                # ---- polish on the full row ------------------------------
                s_t = None
                for t in range(N_FULL_ITERS):
                    cj = junkp.tile([P, SEQ], BF16, tag="junk")
                    s_t = scalars.tile([P, 1], F32, tag="s")
                    nc.vector.tensor_scalar(out=cj[:], in0=e_t[:],
                                            scalar1=c_t[:], scalar2=None,
                                            op0=A.min, op1=A.add,
                                            accum_out=s_t[:])
                    if t < N_FULL_ITERS - 1:
                        c_t = scalars.tile([P, 1], F32, tag="c")
                        nc.vector.tensor_scalar(out=c_t[:], in0=s_t[:],
                                                scalar1=1e-20, scalar2=rk[:],
                                                op0=A.add, op1=A.mult)

                # ---- gamma = min(G * k/s', 1) ----------------------------
                sp = scalars.tile([P, 1], F32, tag="sp")
                nc.vector.tensor_scalar_add(out=sp[:], in0=s_t[:],
                                            scalar1=1e-20)
                rs = scalars.tile([P, 1], F32, tag="rs")
                nc.vector.reciprocal(out=rs[:], in_=sp[:])
                ca = scalars.tile([P, 1], F32, tag="ca")
                nc.vector.tensor_scalar(out=ca[:], in0=rs[:],
                                        scalar1=k_t[:], scalar2=None,
                                        op0=A.mult, op1=A.bypass)
                nc.vector.tensor_scalar(out=sm[:], in0=sm[:], scalar1=ca[:],
                                        scalar2=1.0, op0=A.mult, op1=A.min)
                nc.sync.dma_start(out=gamma_d.ap()[r0:r0 + P, :], in_=sm[:])

    nc.compile()
    return nc


_NC_CACHE = None


def encode_mask(mask: np.ndarray) -> np.ndarray:
    """{0,1} int mask -> additive penalty {-BIG, 0} in bf16."""
    return np.where(np.asarray(mask) == 0, np.float32(-BIG),
                    np.float32(0.0)).astype(ml_dtypes.bfloat16)


def kernel(score: np.ndarray, mask: np.ndarray) -> np.ndarray:
    global _NC_CACHE
    if _NC_CACHE is None:
        _NC_CACHE = build_kernel()
    nc = _NC_CACHE

    maskpen = encode_mask(mask)
    score = np.ascontiguousarray(np.asarray(score, dtype=np.float32))

    in_maps = []
    for i in range(N_CORES):
        sl = slice(i * ROWS_PER_CORE, (i + 1) * ROWS_PER_CORE)
        in_maps.append({
            "score": score[sl],
            "maskbf": np.ascontiguousarray(maskpen[sl]),
        })
    res = run_bass_kernel_spmd(nc, in_maps, core_ids=list(range(N_CORES)))
    out = np.concatenate([res.results[i]["gamma"] for i in range(N_CORES)],
                         axis=0)
    return out.astype(np.float32)



# revision 5
# speedup vs baseline: 3.1165x; 3.1165x over previous
"""Trainium2 Bass kernel for nn_Normalizer (annealed top-k masking normalizer).

Math (see reference): the 20-iteration annealed loop converges to the fixed
point of  c = s(c)/k  with  s(c) = sum_i min(E_i, c),  E_i = exp(sm_i/theta),
theta = 0.3 (constant for the last 12 reference iterations, which forget the
annealing path).  gamma = min(E/c*, 1).

v2 design (multi-engine):
  - host: sm = where(mask==0, -60000, score) in fp16 (single 8MB/core input).
  - ACT: E = exp(sm/theta) bf16 with accum -> s_inf (first fixed-point
    numerator for free); Sign passes count k = 0.1*#unmasked and the Newton
    slope m = #(E >= c) via thresholds in sm-space (needs one tiny Ln).
  - DVE: 4 subsample fixed-point iterations on a strided 1/8 view of E
    (chunks of 16 cols every 128), one full-width min+accum eval, and the
    final gamma = min(E/c, 1) pass (bf16 out).
  - Second Newton eval runs on ACT as Relu+Exp(accum) two-pass so ACT and
    DVE loads balance (~18us/tile each).
  - GpSimd (Pool) executes the [P,1] tensor_tensor scalar updates; Pool does
    not support TensorScalarPtr/imm-scalar forms, so those few run on DVE.
  Newton step 1 uses the exact slope (k - m); step 2 reuses it (stale slope).
  Validated vs the f32 reference: l2 rel err ~3e-3 (gate 2e-2).

Sharding: pure row-parallel, 4096 rows -> 8 cores x 512 rows (4 tiles of
[128, 8192] per core).
"""

import os
import sys

import numpy as np

try:
    import concourse.bass as bass  # noqa: F401
except ImportError:
    sys.path.insert(0, "/opt/trn_rl_repo")
    import concourse.bass as bass  # noqa: F401

import ml_dtypes

import concourse.bacc as bacc
import concourse.tile as tile
from concourse import mybir
from concourse.bass_utils import run_bass_kernel_spmd

F32 = mybir.dt.float32
BF16 = mybir.dt.bfloat16
FP16 = mybir.dt.float16
A = mybir.AluOpType
AF = mybir.ActivationFunctionType

# Problem constants
THETA, P_FRAC = 0.3, 0.1
BSZ, SEQ = 4096, 8192
N_CORES = 8
ROWS_PER_CORE = BSZ // N_CORES          # 512
P = 128                                  # partitions
N_TILES = ROWS_PER_CORE // P             # 4
CHUNK = 16                               # subsample: 16 cols every 128
CHUNK_EVERY = 128
N_CHUNKS = SEQ // CHUNK_EVERY            # 64
SUB = N_CHUNKS * CHUNK                   # 1024
SUB_SCALE = float(SEQ // SUB)            # 8
PEN = -60000.0                           # fp16-representable mask penalty
THRESH = 30000.0                         # mask detect threshold (sign bias)

N_SUB_ITERS = int(os.environ.get("NORM_SUB_ITERS", "4"))
EVAL2 = os.environ.get("NORM_EVAL2", "act")  # 'act' | 'dve'


def _sub_view(ap):
    """[P, SEQ] access pattern -> [P, N_CHUNKS, CHUNK] strided view."""
    return ap.rearrange("p (c l) -> p c l", l=CHUNK_EVERY)[:, :, 0:CHUNK]


def build_kernel():
    nc = bacc.Bacc("TRN2", target_bir_lowering=False, debug=False,
                   num_devices=N_CORES)
    sm_d = nc.dram_tensor("sm", [ROWS_PER_CORE, SEQ], FP16,
                          kind="ExternalInput")
    gamma_d = nc.dram_tensor("gamma", [ROWS_PER_CORE, SEQ], BF16,
                             kind="ExternalOutput")

    with tile.TileContext(nc) as tc:
        with (
            tc.tile_pool(name="smp", bufs=2) as smp,
            tc.tile_pool(name="ep", bufs=2) as ep,
            tc.tile_pool(name="junkp", bufs=2) as junkp,
            tc.tile_pool(name="rp", bufs=2) as rp,
            tc.tile_pool(name="sjp", bufs=2) as sjp,
            tc.tile_pool(name="scal", bufs=8 * N_TILES) as scal,
        ):
            for j in range(N_TILES):
                r0 = j * P
                sm = smp.tile([P, SEQ], FP16, tag="sm")
                nc.sync.dma_start(out=sm[:], in_=sm_d.ap()[r0:r0 + P, :])

                # E = exp(sm/theta) bf16, accum -> s_inf
                e_t = ep.tile([P, SEQ], BF16, tag="E")
                sinf = scal.tile([P, 1], F32, tag="sinf")
                nc.scalar.activation(out=e_t[:], in_=sm[:], func=AF.Exp,
                                     scale=1.0 / THETA, accum_out=sinf[:])
                # k-count: sum sign(sm + 30000) = #unmasked - #masked
                thr = scal.tile([P, 1], F32, tag="thr")
                nc.gpsimd.memset(thr[:], THRESH)
                jk = junkp.tile([P, SEQ], BF16, tag="junk")
                sk = scal.tile([P, 1], F32, tag="sk")
                nc.scalar.activation(out=jk[:], in_=sm[:], func=AF.Sign,
                                     bias=thr[:], accum_out=sk[:])
                # k = 0.1 * (sk + SEQ)/2 ; rk = 1/k ; rk8 = 8/k
                k_t = scal.tile([P, 1], F32, tag="k")
                nc.vector.tensor_scalar(out=k_t[:], in0=sk[:],
                                        scalar1=float(SEQ),
                                        scalar2=P_FRAC / 2.0,
                                        op0=A.add, op1=A.mult)
                rk = scal.tile([P, 1], F32, tag="rk")
                nc.vector.reciprocal(out=rk[:], in_=k_t[:])
                rk8 = scal.tile([P, 1], F32, tag="rk8")
                nc.vector.tensor_scalar(out=rk8[:], in0=rk[:],
                                        scalar1=SUB_SCALE, scalar2=None,
                                        op0=A.mult, op1=A.bypass)
                # c0 = s_inf / k
                c_t = scal.tile([P, 1], F32, tag="c")
                nc.gpsimd.tensor_tensor(out=c_t[:], in0=sinf[:], in1=rk[:],
                                        op=A.mult)

                # --- subsample fixed-point iterations (strided E view) ----
                esv = _sub_view(e_t[:])
                for t in range(N_SUB_ITERS):
                    sj = sjp.tile([P, SUB], BF16, tag="sj")
                    ss = scal.tile([P, 1], F32, tag="ss")
                    nc.vector.tensor_scalar(
                        out=sj[:].rearrange("p (c l) -> p c l", l=CHUNK),
                        in0=esv, scalar1=c_t[:], scalar2=None,
                        op0=A.min, op1=A.add, accum_out=ss[:])
                    c_t = scal.tile([P, 1], F32, tag="c")
                    nc.gpsimd.tensor_tensor(out=c_t[:], in0=ss[:],
                                            in1=rk8[:], op=A.mult)

                # --- Newton step 1: s1 (DVE) + exact slope m1 (ACT) -------
                lnc = scal.tile([P, 1], F32, tag="lnc")
                nc.scalar.activation(out=lnc[:], in_=c_t[:], func=AF.Ln)
                bias1 = scal.tile([P, 1], F32, tag="b1")
                nc.vector.tensor_scalar(out=bias1[:], in0=lnc[:],
                                        scalar1=-THETA, scalar2=None,
                                        op0=A.mult, op1=A.bypass)
                jm = junkp.tile([P, SEQ], BF16, tag="junk")
                sgn = scal.tile([P, 1], F32, tag="sgn")
                nc.scalar.activation(out=jm[:], in_=sm[:], func=AF.Sign,
                                     bias=bias1[:], accum_out=sgn[:])
                j1 = junkp.tile([P, SEQ], BF16, tag="junk")
                s1 = scal.tile([P, 1], F32, tag="s1")
                nc.vector.tensor_scalar(out=j1[:], in0=e_t[:],
                                        scalar1=c_t[:], scalar2=None,
                                        op0=A.min, op1=A.add,
                                        accum_out=s1[:])
                # m1 = (sgn + SEQ)/2 ; denom = max(k - m1, 1) ; rden = 1/denom
                m1 = scal.tile([P, 1], F32, tag="m1")
                nc.vector.tensor_scalar(out=m1[:], in0=sgn[:],
                                        scalar1=float(SEQ), scalar2=0.5,
                                        op0=A.add, op1=A.mult)
                den = scal.tile([P, 1], F32, tag="den")
                nc.gpsimd.tensor_tensor(out=den[:], in0=k_t[:], in1=m1[:],
                                        op=A.subtract)
                den2 = scal.tile([P, 1], F32, tag="den2")
                nc.vector.tensor_scalar(out=den2[:], in0=den[:],
                                        scalar1=1.0, scalar2=None,
                                        op0=A.max, op1=A.bypass)
                rden = scal.tile([P, 1], F32, tag="rden")
                nc.vector.reciprocal(out=rden[:], in_=den2[:])

                def newton_update(c_prev, s_val, tagp):
                    # c_new = c_prev + (s_val - k*c_prev) * rden
                    t1 = scal.tile([P, 1], F32, tag=tagp + "t1")
                    nc.gpsimd.tensor_tensor(out=t1[:], in0=c_prev[:],
                                            in1=k_t[:], op=A.mult)
                    t2 = scal.tile([P, 1], F32, tag=tagp + "t2")
                    nc.gpsimd.tensor_tensor(out=t2[:], in0=s_val[:],
                                            in1=t1[:], op=A.subtract)
                    t3 = scal.tile([P, 1], F32, tag=tagp + "t3")
                    nc.gpsimd.tensor_tensor(out=t3[:], in0=t2[:],
                                            in1=rden[:], op=A.mult)
                    c_new = scal.tile([P, 1], F32, tag=tagp + "c")
                    nc.gpsimd.tensor_tensor(out=c_new[:], in0=c_prev[:],
                                            in1=t3[:], op=A.add)
                    return c_new

                c2 = newton_update(c_t, s1, "n1")

                # --- Newton step 2 (stale slope) --------------------------
                if EVAL2 == "act":
                    # s2 = c2 * sum(exp(min(sm - theta*ln c2, 0)/theta))
                    lnc2 = scal.tile([P, 1], F32, tag="lnc2")
                    nc.scalar.activation(out=lnc2[:], in_=c2[:], func=AF.Ln)
                    r_t = rp.tile([P, SEQ], BF16, tag="r")
                    nc.scalar.activation(out=r_t[:], in_=sm[:], func=AF.Relu,
                                         scale=-1.0 / THETA, bias=lnc2[:])
                    j2 = junkp.tile([P, SEQ], BF16, tag="junk")
                    sc2 = scal.tile([P, 1], F32, tag="sc2")
                    nc.scalar.activation(out=j2[:], in_=r_t[:], func=AF.Exp,
                                         scale=-1.0, accum_out=sc2[:])
                    s2 = scal.tile([P, 1], F32, tag="s2")
                    nc.gpsimd.tensor_tensor(out=s2[:], in0=sc2[:], in1=c2[:],
                                            op=A.mult)
                else:
                    j2 = junkp.tile([P, SEQ], BF16, tag="junk")
                    s2 = scal.tile([P, 1], F32, tag="s2")
                    nc.vector.tensor_scalar(out=j2[:], in0=e_t[:],
                                            scalar1=c2[:], scalar2=None,
                                            op0=A.min, op1=A.add,
                                            accum_out=s2[:])
                c3 = newton_update(c2, s2, "n2")

                # --- gamma = min(E / c3, 1) bf16 --------------------------
                rc = scal.tile([P, 1], F32, tag="rc")
                nc.vector.reciprocal(out=rc[:], in_=c3[:])
                gj = junkp.tile([P, SEQ], BF16, tag="junk")
                nc.vector.tensor_scalar(out=gj[:], in0=e_t[:],
                                        scalar1=rc[:], scalar2=1.0,
                                        op0=A.mult, op1=A.min)
                nc.sync.dma_start(out=gamma_d.ap()[r0:r0 + P, :], in_=gj[:])

    nc.compile()
    return nc


_NC_CACHE = None


def encode_sm(score: np.ndarray, mask: np.ndarray) -> np.ndarray:
    """Pre-masked score in fp16: masked entries -> -60000."""
    sm = np.where(np.asarray(mask) == 0, np.float32(PEN),
                  np.asarray(score, dtype=np.float32))
    return sm.astype(np.float16)


def kernel(score: np.ndarray, mask: np.ndarray) -> np.ndarray:
    global _NC_CACHE
    if _NC_CACHE is None:
        _NC_CACHE = build_kernel()
    nc = _NC_CACHE

    sm = encode_sm(score, mask)
    in_maps = []
    for i in range(N_CORES):
        sl = slice(i * ROWS_PER_CORE, (i + 1) * ROWS_PER_CORE)
        in_maps.append({"sm": np.ascontiguousarray(sm[sl])})
    res = run_bass_kernel_spmd(nc, in_maps, core_ids=list(range(N_CORES)))
    out = np.concatenate([res.results[i]["gamma"] for i in range(N_CORES)],
                         axis=0)
    return out.astype(np.float32)


# revision 7
# speedup vs baseline: 3.6297x; 1.1647x over previous
"""Trainium2 Bass kernel for nn_Normalizer (annealed top-k masking normalizer).

Math (see reference): the 20-iteration annealed loop converges to the fixed
point of  c = s(c)/k,  s(c) = sum_i min(E_i, c),  E_i = exp(sm_i/theta),
theta = 0.3 (the last 12 reference iterations run at constant theta and
forget the annealing path).  gamma = min(E/c*, 1).

v3 design (slope-free over-relaxation, engine-balanced):
  - host: sm = where(mask==0, -60000, score) in fp16 (one 8MB/core input).
  - ACT runs ONLY Exp and Copy (single activation table load):
      E = exp(sm/theta) bf16 with accum -> s_inf;
      k-count: accum of exp(sm/10000): unmasked -> exactly 1.0 (bf16),
      masked -> exactly e^-6, so count = (S - 8192 e^-6)/(1 - e^-6);
      Copy+accum sums a min-pass output (row reduction off the DVE).
  - DVE: subsample fixed-point evals on a strided 1/8 view of E (fused
    min+accum, 1.3us), full-width bf16 min passes (no accum -> 4x mode,
    ~2.2us), tensor_reduce row sums (~4.3us), final gamma pass (~2.35us),
    reciprocals.
  - GpSimd: all [P,1] tensor_tensor scalar updates.
  - updates are over-relaxed with r=2 (contraction |1-r(1-lam)|, lam~0.55):
      c' = (s*rk)^2 / c   -- pure multiplies, no slope passes needed.
    Schedule: c0 = s_inf/k, 3 sub evals, 2 full evals (last update plain).
  Validated: l2 rel err ~2.7e-3 vs f32 reference (gate 2e-2).

Sharding: pure row-parallel, 4096 rows -> 8 cores x 512 rows (4 tiles of
[128, 8192] per core).
"""

import math
import os
import sys

import numpy as np

try:
    import concourse.bass as bass  # noqa: F401
except ImportError:
    sys.path.insert(0, "/opt/trn_rl_repo")
    import concourse.bass as bass  # noqa: F401

import ml_dtypes

import concourse.bacc as bacc
import concourse.tile as tile
from concourse import mybir
from concourse.bass_utils import run_bass_kernel_spmd

F32 = mybir.dt.float32
BF16 = mybir.dt.bfloat16
FP16 = mybir.dt.float16
A = mybir.AluOpType
AF = mybir.ActivationFunctionType

# Problem constants
THETA, P_FRAC = 0.3, 0.1
BSZ, SEQ = 4096, 8192
N_CORES = 8
ROWS_PER_CORE = BSZ // N_CORES          # 512
P = 128                                  # partitions
N_TILES = ROWS_PER_CORE // P             # 4
CHUNK = 16                               # subsample: 16 cols every 128
CHUNK_EVERY = 128
N_CHUNKS = SEQ // CHUNK_EVERY            # 64
SUB = N_CHUNKS * CHUNK                   # 1024
SUB_SCALE = float(SEQ // SUB)            # 8
PEN = -60000.0                           # fp16-representable mask penalty
KTEMP = 10000.0                          # k-count exp temperature
EM6 = math.exp(PEN / KTEMP)              # e^-6
KC1 = -float(SEQ) * EM6                  # count = (S + KC1) * KC2 / P_FRAC...
KC2 = P_FRAC / (1.0 - EM6)               # k = (S + KC1) * KC2

N_SUB_ITERS = int(os.environ.get("NORM_SUB_ITERS", "3"))
S1_MODE = os.environ.get("NORM_S1", "tred")   # 'tred' | 'fused'
S2_MODE = os.environ.get("NORM_S2", "alt")    # 'alt' | 'act' | 'tred'


def _sub_view(ap):
    """[P, SEQ] access pattern -> [P, N_CHUNKS, CHUNK] strided view."""
    return ap.rearrange("p (c l) -> p c l", l=CHUNK_EVERY)[:, :, 0:CHUNK]


def build_kernel():
    nc = bacc.Bacc("TRN2", target_bir_lowering=False, debug=False,
                   num_devices=N_CORES)
    sm_d = nc.dram_tensor("sm", [ROWS_PER_CORE, SEQ], FP16,
                          kind="ExternalInput")
    gamma_d = nc.dram_tensor("gamma", [ROWS_PER_CORE, SEQ], BF16,
                             kind="ExternalOutput")

    with tile.TileContext(nc) as tc:
        with (
            tc.tile_pool(name="smp", bufs=2) as smp,
            tc.tile_pool(name="ep", bufs=2) as ep,
            tc.tile_pool(name="junkp", bufs=3) as junkp,
            tc.tile_pool(name="sjp", bufs=2) as sjp,
            tc.tile_pool(name="scal", bufs=8 * N_TILES) as scal,
        ):
            for j in range(N_TILES):
                r0 = j * P
                sm = smp.tile([P, SEQ], FP16, tag="sm")
                nc.sync.dma_start(out=sm[:], in_=sm_d.ap()[r0:r0 + P, :])

                # E = exp(sm/theta) bf16, accum -> s_inf
                e_t = ep.tile([P, SEQ], BF16, tag="E")
                sinf = scal.tile([P, 1], F32, tag="sinf")
                nc.scalar.activation(out=e_t[:], in_=sm[:], func=AF.Exp,
                                     scale=1.0 / THETA, accum_out=sinf[:])
                # k-count: accum exp(sm/10000): 1.0 per unmasked, e^-6 masked
                jk = junkp.tile([P, SEQ], BF16, tag="junk")
                sc_t = scal.tile([P, 1], F32, tag="sc")
                nc.scalar.activation(out=jk[:], in_=sm[:], func=AF.Exp,
                                     scale=1.0 / KTEMP, accum_out=sc_t[:])
                k_t = scal.tile([P, 1], F32, tag="k")
                nc.vector.tensor_scalar(out=k_t[:], in0=sc_t[:],
                                        scalar1=KC1, scalar2=KC2,
                                        op0=A.add, op1=A.mult)
                rk = scal.tile([P, 1], F32, tag="rk")
                nc.vector.reciprocal(out=rk[:], in_=k_t[:])
                rk8 = scal.tile([P, 1], F32, tag="rk8")
                nc.vector.tensor_scalar(out=rk8[:], in0=rk[:],
                                        scalar1=SUB_SCALE, scalar2=None,
                                        op0=A.mult, op1=A.bypass)
                # c0 = s_inf / k
                c_t = scal.tile([P, 1], F32, tag="c")
                nc.gpsimd.tensor_tensor(out=c_t[:], in0=sinf[:], in1=rk[:],
                                        op=A.mult)

                def relax_update(c_prev, s_val, rk_ap, tagp):
                    # c_new = (s_val*rk)^2 / c_prev   (r=2 over-relaxation)
                    u = scal.tile([P, 1], F32, tag=tagp + "u")
                    nc.gpsimd.tensor_tensor(out=u[:], in0=s_val[:],
                                            in1=rk_ap[:], op=A.mult)
                    u2 = scal.tile([P, 1], F32, tag=tagp + "u2")
                    nc.gpsimd.tensor_tensor(out=u2[:], in0=u[:], in1=u[:],
                                            op=A.mult)
                    rcp = scal.tile([P, 1], F32, tag=tagp + "rcp")
                    nc.vector.reciprocal(out=rcp[:], in_=c_prev[:])
                    c_new = scal.tile([P, 1], F32, tag=tagp + "c")
                    nc.gpsimd.tensor_tensor(out=c_new[:], in0=u2[:],
                                            in1=rcp[:], op=A.mult)
                    return c_new

                # --- subsample over-relaxed evals (strided E view) --------
                esv = _sub_view(e_t[:])
                for t in range(N_SUB_ITERS):
                    sj = sjp.tile([P, SUB], BF16, tag="sj")
                    ss = scal.tile([P, 1], F32, tag="ss")
                    nc.vector.tensor_scalar(
                        out=sj[:].rearrange("p (c l) -> p c l", l=CHUNK),
                        in0=esv, scalar1=c_t[:], scalar2=None,
                        op0=A.min, op1=A.add, accum_out=ss[:])
                    c_t = relax_update(c_t, ss, rk8, f"s{t}")

                # --- full eval 1 (relaxed update) -------------------------
                if S1_MODE == "tred":
                    j1 = junkp.tile([P, SEQ], BF16, tag="junk")
                    nc.vector.tensor_scalar(out=j1[:], in0=e_t[:],
                                            scalar1=c_t[:], scalar2=None,
                                            op0=A.min, op1=A.bypass)
                    s1 = scal.tile([P, 1], F32, tag="s1")
                    nc.vector.tensor_reduce(out=s1[:], in_=j1[:],
                                            axis=mybir.AxisListType.X,
                                            op=A.add)
                else:
                    j1 = junkp.tile([P, SEQ], BF16, tag="junk")
                    s1 = scal.tile([P, 1], F32, tag="s1")
                    nc.vector.tensor_scalar(out=j1[:], in0=e_t[:],
                                            scalar1=c_t[:], scalar2=None,
                                            op0=A.min, op1=A.add,
                                            accum_out=s1[:])
                c2 = relax_update(c_t, s1, rk, "n1")

                # --- full eval 2 (plain update: c3 = s2/k) ----------------
                s2_on_act = (S2_MODE == "act" or
                             (S2_MODE == "alt" and j % 2 == 0))
                j2 = junkp.tile([P, SEQ], BF16, tag="junk")
                s2 = scal.tile([P, 1], F32, tag="s2")
                if s2_on_act:
                    nc.vector.tensor_scalar(out=j2[:], in0=e_t[:],
                                            scalar1=c2[:], scalar2=None,
                                            op0=A.min, op1=A.bypass)
                    jc = junkp.tile([P, SEQ], BF16, tag="junk")
                    nc.scalar.activation(out=jc[:], in_=j2[:], func=AF.Copy,
                                         accum_out=s2[:])
                else:
                    jmin = junkp.tile([P, SEQ], BF16, tag="junk")
                    nc.vector.tensor_scalar(out=jmin[:], in0=e_t[:],
                                            scalar1=c2[:], scalar2=None,
                                            op0=A.min, op1=A.bypass)
                    nc.vector.tensor_reduce(out=s2[:], in_=jmin[:],
                                            axis=mybir.AxisListType.X,
                                            op=A.add)
                c3 = scal.tile([P, 1], F32, tag="c3")
                nc.gpsimd.tensor_tensor(out=c3[:], in0=s2[:], in1=rk[:],
                                        op=A.mult)

                # --- gamma = min(E / c3, 1) bf16 --------------------------
                rc = scal.tile([P, 1], F32, tag="rc")
                nc.vector.reciprocal(out=rc[:], in_=c3[:])
                gj = junkp.tile([P, SEQ], BF16, tag="junk")
                nc.vector.tensor_scalar(out=gj[:], in0=e_t[:],
                                        scalar1=rc[:], scalar2=1.0,
                                        op0=A.mult, op1=A.min)
                nc.sync.dma_start(out=gamma_d.ap()[r0:r0 + P, :], in_=gj[:])

    nc.compile()
    return nc


_NC_CACHE = None


def encode_sm(score: np.ndarray, mask: np.ndarray) -> np.ndarray:
    """Pre-masked score in fp16: masked entries -> -60000."""
    sm = np.where(np.asarray(mask) == 0, np.float32(PEN),
                  np.asarray(score, dtype=np.float32))
    return sm.astype(np.float16)


def kernel(score: np.ndarray, mask: np.ndarray) -> np.ndarray:
    global _NC_CACHE
    if _NC_CACHE is None:
        _NC_CACHE = build_kernel()
    nc = _NC_CACHE

    sm = encode_sm(score, mask)
    in_maps = []
    for i in range(N_CORES):
        sl = slice(i * ROWS_PER_CORE, (i + 1) * ROWS_PER_CORE)
        in_maps.append({"sm": np.ascontiguousarray(sm[sl])})
    res = run_bass_kernel_spmd(nc, in_maps, core_ids=list(range(N_CORES)))
    out = np.concatenate([res.results[i]["gamma"] for i in range(N_CORES)],
                         axis=0)
    return out.astype(np.float32)


# revision 10
# speedup vs baseline: 5.1506x; 1.4190x over previous
"""Trainium2 Bass kernel for nn_Normalizer (annealed top-k masking normalizer).

Math (see reference): the 20-iteration annealed loop converges to the fixed
point of  c = s(c)/k,  s(c) = sum_i min(E_i, c),  E_i = exp(sm_i/theta),
theta = 0.3 (the last 12 reference iterations run at constant theta and
forget the annealing path).  gamma = min(E/c*, 1).

v5 design (latency-shaped pipeline, slope-free r=2 relaxation):
  - host: sm = where(mask==0, -60000, score) in fp16 (one 8MB/core input).
  - ACT queue: E0,k0,E1,k1,copy0,E2,k2,copy1,E3,k3,copy2 -- only Exp/Copy
    (one table load).  E = exp(sm/theta) bf16 + accum s_inf.  Exact
    k-count = accum of exp(sm/10000) (unmasked -> exactly 1.0 in bf16,
    masked -> exactly e^-6).  Copy+accum row-sums min-pass outputs for
    tiles 0-2; tile 3's second eval is a fused DVE min+accum so the last
    row-sum does not serialize on ACT at the tail.
  - DVE per-tile chain depends only on E(j): subsample count -> k_hat
    (exact k arrives later from ACT and is used from the first full eval
    onward), c0 = s_inf/k_hat, 2 subsample evals, fused full eval s1,
    relaxed update, 4x-mode bf16 min pass for s2.  All [P,1] scalar math
    stays on DVE (AP-scalar tensor_scalar + reciprocal) -- no cross-engine
    ping-pong.
  - updates over-relaxed with r=2 (contraction |1-2(1-lam)|, lam~0.55):
      c' = (s*rk)^2/c  -- pure multiplies.
  Validated: l2 rel err ~1.5e-3 vs f32 reference (gate 2e-2).

Sharding: pure row-parallel, 4096 rows -> 8 cores x 512 rows (4 tiles of
[128, 8192] per core).
"""

import math
import os
import sys

import numpy as np

try:
    import concourse.bass as bass  # noqa: F401
except ImportError:
    sys.path.insert(0, "/opt/trn_rl_repo")
    import concourse.bass as bass  # noqa: F401

import ml_dtypes

import concourse.bacc as bacc
import concourse.tile as tile
from concourse import mybir
from concourse.bass_utils import run_bass_kernel_spmd

F32 = mybir.dt.float32
BF16 = mybir.dt.bfloat16
FP16 = mybir.dt.float16
A = mybir.AluOpType
AF = mybir.ActivationFunctionType

# Problem constants
THETA, P_FRAC = 0.3, 0.1
BSZ, SEQ = 4096, 8192
N_CORES = 8
ROWS_PER_CORE = BSZ // N_CORES          # 512
P = 128                                  # partitions
N_TILES = ROWS_PER_CORE // P             # 4
CHUNK = 16                               # subsample: 16 cols every 128
CHUNK_EVERY = 128
N_CHUNKS = SEQ // CHUNK_EVERY            # 64
SUB = N_CHUNKS * CHUNK                   # 1024
SUB_SCALE = float(SEQ // SUB)            # 8
PEN = -60000.0                           # fp16-representable mask penalty
KTEMP = 10000.0                          # k-count exp temperature
EM6 = math.exp(PEN / KTEMP)              # e^-6
KC1 = -float(SEQ) * EM6
KC2 = P_FRAC / (1.0 - EM6)               # k = (S + KC1) * KC2

N_SUB_ITERS = int(os.environ.get("NORM_SUB_ITERS", "2"))


def _sub_view(ap):
    """[P, SEQ] access pattern -> [P, N_CHUNKS, CHUNK] strided view."""
    return ap.rearrange("p (c l) -> p c l", l=CHUNK_EVERY)[:, :, 0:CHUNK]


def build_kernel():
    nc = bacc.Bacc("TRN2", target_bir_lowering=False, debug=False,
                   num_devices=N_CORES)
    sm_d = nc.dram_tensor("sm", [ROWS_PER_CORE, SEQ], FP16,
                          kind="ExternalInput")
    gamma_d = nc.dram_tensor("gamma", [ROWS_PER_CORE, SEQ], BF16,
                             kind="ExternalOutput")

    NT = N_TILES
    with tile.TileContext(nc) as tc:
        with (
            tc.tile_pool(name="smp", bufs=1) as smp,
            tc.tile_pool(name="ep", bufs=1) as ep,
            tc.tile_pool(name="j2p", bufs=1) as j2p,
            tc.tile_pool(name="gjp", bufs=1) as gjp,
            tc.tile_pool(name="jkp", bufs=1) as jkp,
            tc.tile_pool(name="j1p", bufs=1) as j1p,
            tc.tile_pool(name="sjp", bufs=1) as sjp,
            tc.tile_pool(name="scal", bufs=1) as scal,
        ):
            # shared write-only junk outputs
            jk = jkp.tile([P, SEQ], BF16, name="jka", tag="jka")   # ACT junk
            j1 = j1p.tile([P, SEQ], BF16, name="j1d", tag="j1d")   # DVE junk
            sj = sjp.tile([P, SUB], BF16, name="sjd", tag="sjd")   # sub junk

            sm = [None] * NT
            e_t = [None] * NT
            sinf = [None] * NT
            sc_t = [None] * NT
            rk = [None] * NT
            rk2 = [None] * NT
            c_t = [None] * NT
            j2 = [None] * NT
            s2 = [None] * NT

            def ts(out, in0, s1v, s2v, op0, op1=A.bypass, accum=None):
                nc.vector.tensor_scalar(out=out, in0=in0, scalar1=s1v,
                                        scalar2=s2v, op0=op0, op1=op1,
                                        accum_out=accum)

            def new_scal(nm):
                return scal.tile([P, 1], F32, name=nm, tag=nm)

            def dve_relax(j, c_prev, s_val, rksq, tagp):
                """c_new = (s_val^2 * rksq) / c_prev, all on DVE."""
                u2 = new_scal(f"{tagp}u2_{j}")
                ts(u2[:], s_val[:], s_val[:], rksq[:], A.mult, A.mult)
                rcp = new_scal(f"{tagp}rcp_{j}")
                nc.vector.reciprocal(out=rcp[:], in_=c_prev[:])
                c_new = new_scal(f"{tagp}c_{j}")
                ts(c_new[:], u2[:], rcp[:], None, A.mult)
                return c_new

            def emit_act(j):
                e_t[j] = ep.tile([P, SEQ], BF16, name=f"E{j}", tag=f"E{j}")
                sinf[j] = new_scal(f"sinf{j}")
                nc.scalar.activation(out=e_t[j][:], in_=sm[j][:],
                                     func=AF.Exp, scale=1.0 / THETA,
                                     accum_out=sinf[j][:])
                sc_t[j] = new_scal(f"sc{j}")
                nc.scalar.activation(out=jk[:], in_=sm[j][:], func=AF.Exp,
                                     scale=1.0 / KTEMP,
                                     accum_out=sc_t[j][:])

            def emit_chain(j):
                # subsample count -> k_hat consts (exact k not needed yet)
                cs = new_scal(f"cs{j}")
                ts(sj[:].rearrange("p (c l) -> p c l", l=CHUNK),
                   _sub_view(e_t[j][:]), 0.0, None, A.is_gt, A.add,
                   accum=cs[:])
                kh = new_scal(f"kh{j}")
                ts(kh[:], cs[:], SUB_SCALE * P_FRAC, None, A.mult)
                rkh = new_scal(f"rkh{j}")
                nc.vector.reciprocal(out=rkh[:], in_=kh[:])
                rkh2 = new_scal(f"rkh2{j}")
                ts(rkh2[:], rkh[:], rkh[:], None, A.mult)
                r8kh2 = new_scal(f"r8kh2{j}")
                ts(r8kh2[:], rkh2[:], SUB_SCALE * SUB_SCALE, None, A.mult)
                c_t[j] = new_scal(f"c0_{j}")
                ts(c_t[j][:], sinf[j][:], rkh[:], None, A.mult)
                # subsample relaxed evals
                for t in range(N_SUB_ITERS):
                    ss = new_scal(f"ss{t}_{j}")
                    ts(sj[:].rearrange("p (c l) -> p c l", l=CHUNK),
                       _sub_view(e_t[j][:]), c_t[j][:], None, A.min, A.add,
                       accum=ss[:])
                    c_t[j] = dve_relax(j, c_t[j], ss, r8kh2, f"s{t}")
                # full eval 1 (fused min+accum)
                s1 = new_scal(f"s1_{j}")
                ts(j1[:], e_t[j][:], c_t[j][:], None, A.min, A.add,
                   accum=s1[:])
                # exact k consts (ACT kexp(j) accum is long since ready)
                k_t = new_scal(f"k{j}")
                ts(k_t[:], sc_t[j][:], KC1, KC2, A.add, A.mult)
                rk[j] = new_scal(f"rk{j}")
                nc.vector.reciprocal(out=rk[j][:], in_=k_t[:])
                rk2[j] = new_scal(f"rk2{j}")
                ts(rk2[j][:], rk[j][:], rk[j][:], None, A.mult)
                c_t[j] = dve_relax(j, c_t[j], s1, rk2[j], "n1")
                # second eval: min pass (4x) summed by ACT copy (tiles 0-2)
                # or fused on DVE (tile 3)
                s2[j] = new_scal(f"s2_{j}")
                if j < NT - 1:
                    j2[j] = j2p.tile([P, SEQ], BF16, name=f"j2_{j % 2}",
                                     tag=f"j2_{j % 2}")
                    ts(j2[j][:], e_t[j][:], c_t[j][:], None, A.min)
                else:
                    ts(j1[:], e_t[j][:], c_t[j][:], None, A.min, A.add,
                       accum=s2[j][:])

            def emit_copy(j):
                nc.scalar.activation(out=jk[:], in_=j2[j][:], func=AF.Copy,
                                     accum_out=s2[j][:])

            def emit_tail(j):
                c3 = dve_relax(j, c_t[j], s2[j], rk2[j], "n2")
                rc = new_scal(f"rc{j}")
                nc.vector.reciprocal(out=rc[:], in_=c3[:])
                gj = gjp.tile([P, SEQ], BF16, name=f"gj{j % 2}",
                              tag=f"gj{j % 2}")
                ts(gj[:], e_t[j][:], rc[:], 1.0, A.mult, A.min)
                nc.sync.dma_start(out=gamma_d.ap()[j * P:(j + 1) * P, :],
                                  in_=gj[:])

            # input DMAs
            for j in range(NT):
                sm[j] = smp.tile([P, SEQ], FP16, name=f"sm{j % 2}",
                                 tag=f"sm{j % 2}")
                nc.sync.dma_start(out=sm[j][:],
                                  in_=sm_d.ap()[j * P:(j + 1) * P, :])

            # interleaved emission: ACT gets E0,k0,E1,k1,cp0,E2,k2,cp1,
            # E3,k3,cp2; DVE gets chain0..chain3 then tails.
            emit_act(0)
            emit_chain(0)
            emit_act(1)
            emit_chain(1)
            emit_copy(0)
            emit_act(2)
            emit_chain(2)
            emit_copy(1)
            emit_act(3)
            emit_chain(3)
            emit_copy(2)
            for j in range(NT):
                emit_tail(j)

    nc.compile()
    return nc


_NC_CACHE = None


def encode_sm(score: np.ndarray, mask: np.ndarray) -> np.ndarray:
    """Pre-masked score in fp16: masked entries -> -60000."""
    sm = np.where(np.asarray(mask) == 0, np.float32(PEN),
                  np.asarray(score, dtype=np.float32))
    return sm.astype(np.float16)


def kernel(score: np.ndarray, mask: np.ndarray) -> np.ndarray:
    global _NC_CACHE
    if _NC_CACHE is None:
        _NC_CACHE = build_kernel()
    nc = _NC_CACHE

    sm = encode_sm(score, mask)
    in_maps = []
    for i in range(N_CORES):
        sl = slice(i * ROWS_PER_CORE, (i + 1) * ROWS_PER_CORE)
        in_maps.append({"sm": np.ascontiguousarray(sm[sl])})
    res = run_bass_kernel_spmd(nc, in_maps, core_ids=list(range(N_CORES)))
    out = np.concatenate([res.results[i]["gamma"] for i in range(N_CORES)],
                         axis=0)
    return out.astype(np.float32)
